# revision 21
# baseline (speedup 1.0000x reference)
"""AWBNet (wo R2) Trainium2 kernel.

Math (per sample b):
  m = reshape(relu(hist_flat @ W1 + b1) @ W2 + b2, [9, 3])
  feats(px) = [r, g, b, r^2, g^2, b^2, rg, rb, gb]
  y[px, c] = sum_k feats[px, k] * m[k, c]

Device strategy (8 cores, pure data parallel over batch, 2 samples/core):
  * Tiny MLP on TensorE in fp32 with natural layouts (host only re-packs
    histogram / b1 so no on-device transposes are needed).
  * Per-pixel einsum on VectorE/ScalarE in fp16 using the square basis
    {R, G, B, R^2, G^2, B^2, (R+G)^2, (R+B)^2, (G+B)^2}; the coefficient
    change (rg = ((R+G)^2 - R^2 - G^2)/2 etc.) is folded into W2/b2 on the
    host (pure linear re-parameterization of the weights, no data compute).
  * m-coefficients are broadcast to all 128 partitions by a fused
    matmul whose lhsT is a stride-0 (broadcast) column of featT; they are
    then per-partition scalars for the per-pixel products.
  * Per-pixel: ScalarE deinterleaves (stride-3 fp32 -> dense fp16) and
    squares; products m_k*F_k split DVE tensor_scalar (2x) / ACT
    scale-copies to balance the engines; DVE tt-add trees combine, the
    last add writing the stride-3 fp32 output view directly.
  * Three DMA queues in parallel: W1 stream + late x tiles on SWDGE
    (with fp32->fp16 cast), x0/x1 + y stores on the SP HWDGE ring, small
    setup DMAs on the ACT HWDGE ring.
"""

import sys

import numpy as np

for _p in ("/opt/trn_rl_repo",):
    if _p not in sys.path:
        sys.path.insert(0, _p)

import concourse.bacc as bacc
import concourse.mybir as mybir
import concourse.tile as tile
from concourse import bass_utils

# ---- problem constants (hardcoded per contract) ----
N_CORES = 8
B, H, W, C = 16, 512, 512, 3
SPC = B // N_CORES  # samples per core = 2
PX_SAMPLE = H * W  # 262144
PX_CORE = SPC * PX_SAMPLE  # 524288
P = 128
LANE_PX = PX_CORE // P  # 4096 pixels per partition per core
T = 1024  # pixels per partition per tile
NTILES = LANE_PX // T  # 4
TILES_PER_SAMPLE = NTILES // SPC  # 2

HIST = 3 * 64 * 64  # 12288
HID = 256
MOUT = 27
KT = HIST // P  # 96 k-tiles
MT = HID // P  # 2 m-tiles
W1_CH = 12  # k-tiles per W1 DMA chunk (12 * 128KB = 1.5MB)
KT_SH = KT // N_CORES  # 12 k-tiles of W1 per core (K-sharded MLP + AllReduce)

F16 = mybir.dt.float16
BF16 = mybir.dt.bfloat16
PLDT = mybir.dt.float16
F32 = mybir.dt.float32
MULT = mybir.AluOpType.mult
ADD = mybir.AluOpType.add
AF = mybir.ActivationFunctionType

_CACHE = {}


def _coeff_transform():
    """T27 so that m' = m_flat @ T27.T gives coefficients for the square
    basis [R,G,B,R2,G2,B2,(R+G)^2,(R+B)^2,(G+B)^2]."""
    T9 = np.zeros((9, 9), dtype=np.float64)
    for i in range(3):  # R,G,B linear terms pass through
        T9[i, i] = 1.0
    # new squares: old squares minus half the relevant cross terms
    # old order: 3=r2,4=g2,5=b2,6=rg,7=rb,8=gb
    T9[3, 3] = 1.0
    T9[3, 6] = -0.5
    T9[3, 7] = -0.5
    T9[4, 4] = 1.0
    T9[4, 6] = -0.5
    T9[4, 8] = -0.5
    T9[5, 5] = 1.0
    T9[5, 7] = -0.5
    T9[5, 8] = -0.5
    T9[6, 6] = 0.5  # (R+G)^2 coeff = rg/2
    T9[7, 7] = 0.5
    T9[8, 8] = 0.5
    T27 = np.zeros((27, 27), dtype=np.float64)
    for c in range(3):
        for kn in range(9):
            for ko in range(9):
                T27[3 * kn + c, 3 * ko + c] = T9[kn, ko]
    return T27


def _build():
    nc = bacc.Bacc(
        "TRN2", target_bir_lowering=False, debug=False, num_devices=N_CORES
    )

    x_d = nc.dram_tensor("x_core", [NTILES, P, T * C], F32, kind="ExternalInput")
    hp_d = nc.dram_tensor("h_packed", [P, KT * SPC], F32, kind="ExternalInput")
    w1_d = nc.dram_tensor("w1", [KT, P, HID], F32, kind="ExternalInput")
    b1_d = nc.dram_tensor("b1_rep", [SPC, HID], F32, kind="ExternalInput")
    w2_d = nc.dram_tensor("w2p", [MT, P, MOUT], F32, kind="ExternalInput")
    b2_d = nc.dram_tensor("b2bc", [P, SPC * MOUT], F32, kind="ExternalInput")
    eye_d = nc.dram_tensor("eye2", [SPC, SPC], F32, kind="ExternalInput")
    y_d = nc.dram_tensor("y_core", [NTILES, P, T * C], F32, kind="ExternalOutput")

    with tile.TileContext(nc) as tc:
        with (
            tc.tile_pool(name="mlp", bufs=1) as mlp_pool,
            tc.tile_pool(name="w1s", bufs=3) as w1_pool,
            tc.tile_pool(name="px32", bufs=2) as px32_pool,
            tc.tile_pool(name="pl16", bufs=2) as plane_pool,
            tc.tile_pool(name="ps", bufs=1, space="PSUM") as psum_pool,
        ):
            # ---------------- MLP (TensorE) ----------------
            hp_sb = mlp_pool.tile([P, KT * SPC], F16, tag="hp", name="hp")
            nc.gpsimd.dma_start(out=hp_sb, in_=hp_d[:, :])
            b1_sb = mlp_pool.tile([SPC, HID], F32, tag="b1", name="b1")
            nc.scalar.dma_start(out=b1_sb, in_=b1_d[:, :])
            w2_sb = mlp_pool.tile([P, MT, MOUT], F32, tag="w2", name="w2")
            nc.scalar.dma_start(out=w2_sb, in_=w2_d.rearrange("m p n -> p m n"))
            b2_sb = mlp_pool.tile([P, SPC * MOUT], F32, tag="b2", name="b2")
            nc.scalar.dma_start(out=b2_sb, in_=b2_d[:, :])
            eye_sb = mlp_pool.tile([SPC, SPC], F32, tag="eye", name="eye")
            nc.scalar.dma_start(out=eye_sb, in_=eye_d[:, :])

            # feat = h @ W1: lhsT = h-slices [128, 2] (cheap weight loads),
            # rhs = W1 k-tiles [128, 256] -> psum [2, 256] accumulated.
            feat_ps = psum_pool.tile([SPC, HID], F32, tag="featps", name="featps")
            for kc in range(KT // W1_CH):
                w1_sb = w1_pool.tile([P, W1_CH, HID], F16, tag="w1c", name="w1c")
                nc.gpsimd.dma_start(
                    out=w1_sb,
                    in_=w1_d[kc * W1_CH : (kc + 1) * W1_CH].rearrange(
                        "k p n -> p k n"
                    ),
                )
                for kk in range(W1_CH):
                    k = kc * W1_CH + kk
                    nc.tensor.matmul(
                        feat_ps,
                        hp_sb[:, k * SPC : (k + 1) * SPC],
                        w1_sb[:, kk, :],
                        start=(k == 0),
                        stop=(k == KT - 1),
                    )

            # relu(feat + b1) on DVE (b1 lives on the free dim here)
            feat_sb = mlp_pool.tile([SPC, HID], F32, tag="featsb", name="featsb")
            nc.vector.tensor_add(feat_sb, feat_ps, b1_sb)
            feat_r = mlp_pool.tile([SPC, HID], F32, tag="featr", name="featr")
            nc.vector.tensor_scalar(
                feat_r, feat_sb, 0.0, None, mybir.AluOpType.max
            )

            # transpose feat [2, 256] -> featT tiles [128, 2] via PE
            featT_sb = []
            for mt in range(MT):
                ft_ps = psum_pool.tile(
                    [P, SPC], F32, tag=f"ftps{mt}", name=f"ftps{mt}"
                )
                nc.tensor.transpose(
                    ft_ps, feat_r[:, mt * P : (mt + 1) * P], eye_sb
                )
                ft_sb = mlp_pool.tile(
                    [P, SPC], F32, tag=f"ftsb{mt}", name=f"ftsb{mt}"
                )
                nc.vector.tensor_copy(ft_sb, ft_ps)
                featT_sb.append(ft_sb)

            # fused m-matmul + partition-broadcast: a stride-0 lhsT column
            # makes every output partition compute m[s] = featT[:, s] @ W2'.
            mb_ps = psum_pool.tile([P, SPC * MOUT], F32, tag="mbps", name="mbps")
            for s in range(SPC):
                for mt in range(MT):
                    nc.tensor.matmul(
                        mb_ps[:, s * MOUT : (s + 1) * MOUT],
                        featT_sb[mt][:, s : s + 1].broadcast_to([P, P]),
                        w2_sb[:, mt, :],
                        start=(mt == 0),
                        stop=(mt == MT - 1),
                    )
            mscal = mlp_pool.tile([P, SPC * MOUT], F32, tag="mscal", name="mscal")
            nc.vector.tensor_add(mscal, mb_ps, b2_sb)

            # ---------------- pixel path ----------------
            for t in range(NTILES):
                s = t // TILES_PER_SAMPLE

                def ms(k, c, s=s):
                    j = s * MOUT + 3 * k + c
                    return mscal[:, j : j + 1]

                x32 = px32_pool.tile([P, T, C], F32, tag="x32", name="x32")
                x_dma = nc.sync if t < 2 else nc.gpsimd
                x_dma.dma_start(out=x32, in_=x_d[t].rearrange("p (t c) -> p t c", c=C))

                # deinterleave + cast to fp16 (ACT, stride-3 reads)
                rp = plane_pool.tile([P, T], PLDT, tag="rp", name="rp")
                gp = plane_pool.tile([P, T], PLDT, tag="gp", name="gp")
                bp = plane_pool.tile([P, T], PLDT, tag="bp", name="bp")
                nc.scalar.copy(rp, x32[:, :, 0])
                nc.scalar.copy(gp, x32[:, :, 1])
                nc.scalar.copy(bp, x32[:, :, 2])

                # pair sums (DVE fp16 2x)
                srg = plane_pool.tile([P, T], PLDT, tag="srg", name="srg")
                srb = plane_pool.tile([P, T], PLDT, tag="srb", name="srb")
                sgb = plane_pool.tile([P, T], PLDT, tag="sgb", name="sgb")
                nc.vector.tensor_add(srg, rp, gp)
                nc.vector.tensor_add(srb, rp, bp)
                nc.vector.tensor_add(sgb, gp, bp)

                # squares (ACT)
                r2 = plane_pool.tile([P, T], PLDT, tag="r2", name="r2")
                g2 = plane_pool.tile([P, T], PLDT, tag="g2", name="g2")
                b2 = plane_pool.tile([P, T], PLDT, tag="b2", name="b2")
                qrg = plane_pool.tile([P, T], PLDT, tag="qrg", name="qrg")
                qrb = plane_pool.tile([P, T], PLDT, tag="qrb", name="qrb")
                qgb = plane_pool.tile([P, T], PLDT, tag="qgb", name="qgb")
                nc.scalar.square(r2, rp)
                nc.scalar.square(g2, gp)
                nc.scalar.square(b2, bp)
                nc.scalar.square(qrg, srg)
                nc.scalar.square(qrb, srb)
                nc.scalar.square(qgb, sgb)

                basis = [rp, gp, bp, r2, g2, b2, qrg, qrb, qgb]

                y32 = px32_pool.tile([P, T, C], F32, tag="y32", name="y32")
                # products: DVE tensor_scalar (2x) for ACT_K-complement,
                # ACT scale-copies for ACT_K; combine with a tt-add tree.
                ACT_K = (3, 4, 5, 6)  # products computed on ScalarE
                for c in range(C):
                    u = []
                    for k in range(9):
                        uk = plane_pool.tile(
                            [P, T], PLDT, tag=f"u{k}", name=f"u{c}_{k}", 
                        )
                        if k in ACT_K:
                            nc.scalar.mul(uk, basis[k], ms(k, c))
                        else:
                            nc.vector.tensor_scalar(
                                uk, basis[k], ms(k, c), None, MULT
                            )
                        u.append(uk)
                    nc.vector.tensor_add(u[0], u[0], u[1])
                    nc.vector.tensor_add(u[2], u[2], u[3])
                    nc.vector.tensor_add(u[4], u[4], u[5])
                    nc.vector.tensor_add(u[6], u[6], u[7])
                    nc.vector.tensor_add(u[0], u[0], u[2])
                    nc.vector.tensor_add(u[4], u[4], u[6])
                    nc.vector.tensor_add(u[0], u[0], u[8])
                    nc.vector.tensor_add(y32[:, :, c], u[0], u[4])

                nc.sync.dma_start(
                    out=y_d[t].rearrange("p (t c) -> p t c", c=C), in_=y32
                )

    nc.compile()
    return nc


def _prep_inputs(x, histogram, W1, b1, W2, b2):
    """Host-side sharding / layout packing (no arithmetic on data except the
    static linear re-parameterization of the tiny weights W2/b2)."""
    x = np.ascontiguousarray(np.asarray(x, dtype=np.float32))
    hist = np.asarray(histogram, dtype=np.float32).reshape(B, HIST)
    W1 = np.ascontiguousarray(np.asarray(W1, dtype=np.float32))
    b1 = np.asarray(b1, dtype=np.float32)
    W2 = np.asarray(W2, dtype=np.float32)
    b2 = np.asarray(b2, dtype=np.float32)

    T27 = _coeff_transform()
    W2p = np.ascontiguousarray(
        (W2.astype(np.float64) @ T27.T).astype(np.float32).reshape(MT, P, MOUT)
    )
    b2p_flat = (b2.astype(np.float64) @ T27.T).astype(np.float32)

    w1_r = W1.reshape(KT, P, HID)
    b1rep = np.ascontiguousarray(np.broadcast_to(b1, (SPC, HID)))
    b2bc = np.ascontiguousarray(
        np.broadcast_to(np.tile(b2p_flat, SPC), (P, SPC * MOUT))
    )
    eye2 = np.eye(SPC, dtype=np.float32)

    in_maps = []
    for core in range(N_CORES):
        xs = x[core * SPC : (core + 1) * SPC].reshape(-1)
        x_core = np.ascontiguousarray(xs.reshape(NTILES, P, T * C))
        h_core = hist[core * SPC : (core + 1) * SPC]  # [SPC, HIST]
        hp = np.ascontiguousarray(
            h_core.reshape(SPC, KT, P).transpose(2, 1, 0).reshape(P, KT * SPC)
        )
        in_maps.append(
            {
                "x_core": x_core,
                "h_packed": hp,
                "w1": w1_r,
                "b1_rep": b1rep,
                "eye2": eye2,
                "w2p": W2p,
                "b2bc": b2bc,
            }
        )
    return in_maps


def run(trace=False, **inputs):
    if "nc" not in _CACHE:
        _CACHE["nc"] = _build()
    nc = _CACHE["nc"]
    in_maps = _prep_inputs(**inputs)
    res = bass_utils.run_bass_kernel_spmd(
        nc, in_maps, core_ids=list(range(N_CORES)), trace=trace
    )
    outs = np.stack([r["y_core"] for r in res.results])  # [8, NTILES, P, T*C]
    y = outs.reshape(B, H, W, C).astype(np.float32)
    return y, res


def kernel(**inputs) -> np.ndarray:
    y, _ = run(trace=False, **inputs)
    return y


if __name__ == "__main__":
    rng = np.random.default_rng(0)
    ins = {
        "x": rng.random((B, H, W, C), dtype=np.float32),
        "histogram": rng.random((B, 3, 64, 64), dtype=np.float32),
        "W1": (rng.standard_normal((HIST, HID)) / np.sqrt(HIST)).astype(np.float32),
        "b1": np.zeros(HID, np.float32),
        "W2": (rng.standard_normal((HID, MOUT)) / np.sqrt(HID)).astype(np.float32),
        "b2": np.zeros(MOUT, np.float32),
    }
    y = kernel(**ins)
    print("out", y.shape, y.dtype, float(np.abs(y).max()))


# revision 22
# speedup vs baseline: 1.0478x; 1.0478x over previous
"""AWBNet (wo R2) Trainium2 kernel.

Math (per sample b):
  m = reshape(relu(hist_flat @ W1 + b1) @ W2 + b2, [9, 3])
  feats(px) = [r, g, b, r^2, g^2, b^2, rg, rb, gb]
  y[px, c] = sum_k feats[px, k] * m[k, c]

Device strategy (8 cores, pure data parallel over batch, 2 samples/core):
  * Tiny MLP on TensorE in fp32 with natural layouts (host only re-packs
    histogram / b1 so no on-device transposes are needed).
  * Per-pixel einsum on VectorE/ScalarE in fp16 using the square basis
    {R, G, B, R^2, G^2, B^2, (R+G)^2, (R+B)^2, (G+B)^2}; the coefficient
    change (rg = ((R+G)^2 - R^2 - G^2)/2 etc.) is folded into W2/b2 on the
    host (pure linear re-parameterization of the weights, no data compute).
  * m-coefficients are broadcast to all 128 partitions by a fused
    matmul whose lhsT is a stride-0 (broadcast) column of featT; they are
    then per-partition scalars for the per-pixel products.
  * Per-pixel: ScalarE deinterleaves (stride-3 fp32 -> dense fp16) and
    squares; products m_k*F_k split DVE tensor_scalar (2x) / ACT
    scale-copies to balance the engines; DVE tt-add trees combine, the
    last add writing the stride-3 fp32 output view directly.
  * Three DMA queues in parallel: W1 stream + late x tiles on SWDGE
    (with fp32->fp16 cast), x0/x1 + y stores on the SP HWDGE ring, small
    setup DMAs on the ACT HWDGE ring.
"""

import sys

import numpy as np

for _p in ("/opt/trn_rl_repo",):
    if _p not in sys.path:
        sys.path.insert(0, _p)

import concourse.bacc as bacc
import concourse.mybir as mybir
import concourse.tile as tile
from concourse import bass_utils

# ---- problem constants (hardcoded per contract) ----
N_CORES = 8
B, H, W, C = 16, 512, 512, 3
SPC = B // N_CORES  # samples per core = 2
PX_SAMPLE = H * W  # 262144
PX_CORE = SPC * PX_SAMPLE  # 524288
P = 128
LANE_PX = PX_CORE // P  # 4096 pixels per partition per core
T = 1024  # pixels per partition per tile
NTILES = LANE_PX // T  # 4
TILES_PER_SAMPLE = NTILES // SPC  # 2

HIST = 3 * 64 * 64  # 12288
HID = 256
MOUT = 27
KT = HIST // P  # 96 k-tiles
MT = HID // P  # 2 m-tiles
W1_CH = 8  # k-tiles per W1 DMA chunk (8 * 128KB = 1MB)
KT_SH = KT // N_CORES  # 12 k-tiles of W1 per core (K-sharded MLP + AllReduce)

F16 = mybir.dt.float16
BF16 = mybir.dt.bfloat16
PLDT = mybir.dt.float16
F32 = mybir.dt.float32
MULT = mybir.AluOpType.mult
ADD = mybir.AluOpType.add
AF = mybir.ActivationFunctionType

_CACHE = {}


def _coeff_transform():
    """T27 so that m' = m_flat @ T27.T gives coefficients for the square
    basis [R,G,B,R2,G2,B2,(R+G)^2,(R+B)^2,(G+B)^2]."""
    T9 = np.zeros((9, 9), dtype=np.float64)
    for i in range(3):  # R,G,B linear terms pass through
        T9[i, i] = 1.0
    # new squares: old squares minus half the relevant cross terms
    # old order: 3=r2,4=g2,5=b2,6=rg,7=rb,8=gb
    T9[3, 3] = 1.0
    T9[3, 6] = -0.5
    T9[3, 7] = -0.5
    T9[4, 4] = 1.0
    T9[4, 6] = -0.5
    T9[4, 8] = -0.5
    T9[5, 5] = 1.0
    T9[5, 7] = -0.5
    T9[5, 8] = -0.5
    T9[6, 6] = 0.5  # (R+G)^2 coeff = rg/2
    T9[7, 7] = 0.5
    T9[8, 8] = 0.5
    T27 = np.zeros((27, 27), dtype=np.float64)
    for c in range(3):
        for kn in range(9):
            for ko in range(9):
                T27[3 * kn + c, 3 * ko + c] = T9[kn, ko]
    return T27


def _build():
    nc = bacc.Bacc(
        "TRN2", target_bir_lowering=False, debug=False, num_devices=N_CORES
    )

    x_d = nc.dram_tensor("x_core", [NTILES, P, T * C], F32, kind="ExternalInput")
    hp_d = nc.dram_tensor("h_packed", [P, KT * SPC], F32, kind="ExternalInput")
    w1_d = nc.dram_tensor("w1", [KT, P, HID], F32, kind="ExternalInput")
    b1_d = nc.dram_tensor("b1_rep", [SPC, HID], F32, kind="ExternalInput")
    w2_d = nc.dram_tensor("w2p", [MT, P, MOUT], F32, kind="ExternalInput")
    b2_d = nc.dram_tensor("b2bc", [P, SPC * MOUT], F32, kind="ExternalInput")
    eye_d = nc.dram_tensor("eye2", [SPC, SPC], F32, kind="ExternalInput")
    y_d = nc.dram_tensor("y_core", [NTILES, P, T * C], F32, kind="ExternalOutput")

    with tile.TileContext(nc) as tc:
        with (
            tc.tile_pool(name="mlp", bufs=1) as mlp_pool,
            tc.tile_pool(name="w1s", bufs=3) as w1_pool,
            tc.tile_pool(name="px32", bufs=2) as px32_pool,
            tc.tile_pool(name="pl16", bufs=2) as plane_pool,
            tc.tile_pool(name="ps", bufs=1, space="PSUM") as psum_pool,
        ):
            # ---------------- MLP (TensorE) ----------------
            hp_sb = mlp_pool.tile([P, KT * SPC], F16, tag="hp", name="hp")
            nc.gpsimd.dma_start(out=hp_sb, in_=hp_d[:, :])
            b1_sb = mlp_pool.tile([SPC, HID], F32, tag="b1", name="b1")
            nc.scalar.dma_start(out=b1_sb, in_=b1_d[:, :])
            w2_sb = mlp_pool.tile([P, MT, MOUT], F32, tag="w2", name="w2")
            nc.scalar.dma_start(out=w2_sb, in_=w2_d.rearrange("m p n -> p m n"))
            b2_sb = mlp_pool.tile([P, SPC * MOUT], F32, tag="b2", name="b2")
            nc.scalar.dma_start(out=b2_sb, in_=b2_d[:, :])
            eye_sb = mlp_pool.tile([SPC, SPC], F32, tag="eye", name="eye")
            nc.scalar.dma_start(out=eye_sb, in_=eye_d[:, :])

            # feat = h @ W1: lhsT = h-slices [128, 2] (cheap weight loads),
            # rhs = W1 k-tiles [128, 256] -> psum [2, 256] accumulated.
            feat_ps = psum_pool.tile([SPC, HID], F32, tag="featps", name="featps")
            for kc in range(KT // W1_CH):
                w1_sb = w1_pool.tile([P, W1_CH, HID], F16, tag="w1c", name="w1c")
                nc.gpsimd.dma_start(
                    out=w1_sb,
                    in_=w1_d[kc * W1_CH : (kc + 1) * W1_CH].rearrange(
                        "k p n -> p k n"
                    ),
                )
                for kk in range(W1_CH):
                    k = kc * W1_CH + kk
                    nc.tensor.matmul(
                        feat_ps,
                        hp_sb[:, k * SPC : (k + 1) * SPC],
                        w1_sb[:, kk, :],
                        start=(k == 0),
                        stop=(k == KT - 1),
                    )

            # relu(feat + b1) on DVE (b1 lives on the free dim here)
            feat_sb = mlp_pool.tile([SPC, HID], F32, tag="featsb", name="featsb")
            nc.vector.tensor_add(feat_sb, feat_ps, b1_sb)
            feat_r = mlp_pool.tile([SPC, HID], F32, tag="featr", name="featr")
            nc.vector.tensor_scalar(
                feat_r, feat_sb, 0.0, None, mybir.AluOpType.max
            )

            # transpose feat [2, 256] -> featT tiles [128, 2] via PE
            featT_sb = []
            for mt in range(MT):
                ft_ps = psum_pool.tile(
                    [P, SPC], F32, tag=f"ftps{mt}", name=f"ftps{mt}"
                )
                nc.tensor.transpose(
                    ft_ps, feat_r[:, mt * P : (mt + 1) * P], eye_sb
                )
                ft_sb = mlp_pool.tile(
                    [P, SPC], F32, tag=f"ftsb{mt}", name=f"ftsb{mt}"
                )
                nc.vector.tensor_copy(ft_sb, ft_ps)
                featT_sb.append(ft_sb)

            # fused m-matmul + partition-broadcast: a stride-0 lhsT column
            # makes every output partition compute m[s] = featT[:, s] @ W2'.
            mb_ps = psum_pool.tile([P, SPC * MOUT], F32, tag="mbps", name="mbps")
            for s in range(SPC):
                for mt in range(MT):
                    nc.tensor.matmul(
                        mb_ps[:, s * MOUT : (s + 1) * MOUT],
                        featT_sb[mt][:, s : s + 1].broadcast_to([P, P]),
                        w2_sb[:, mt, :],
                        start=(mt == 0),
                        stop=(mt == MT - 1),
                    )
            mscal = mlp_pool.tile([P, SPC * MOUT], F32, tag="mscal", name="mscal")
            nc.vector.tensor_add(mscal, mb_ps, b2_sb)

            # ---------------- pixel path ----------------
            for t in range(NTILES):
                s = t // TILES_PER_SAMPLE

                def ms(k, c, s=s):
                    j = s * MOUT + 3 * k + c
                    return mscal[:, j : j + 1]

                x32 = px32_pool.tile([P, T, C], F32, tag="x32", name="x32")
                x_dma = nc.sync if t < 2 else nc.gpsimd
                x_dma.dma_start(out=x32, in_=x_d[t].rearrange("p (t c) -> p t c", c=C))

                # deinterleave + cast to fp16 (ACT, stride-3 reads)
                rp = plane_pool.tile([P, T], PLDT, tag="rp", name="rp")
                gp = plane_pool.tile([P, T], PLDT, tag="gp", name="gp")
                bp = plane_pool.tile([P, T], PLDT, tag="bp", name="bp")
                nc.scalar.copy(rp, x32[:, :, 0])
                nc.scalar.copy(gp, x32[:, :, 1])
                nc.scalar.copy(bp, x32[:, :, 2])

                # pair sums (DVE fp16 2x)
                srg = plane_pool.tile([P, T], PLDT, tag="srg", name="srg")
                srb = plane_pool.tile([P, T], PLDT, tag="srb", name="srb")
                sgb = plane_pool.tile([P, T], PLDT, tag="sgb", name="sgb")
                nc.vector.tensor_add(srg, rp, gp)
                nc.vector.tensor_add(srb, rp, bp)
                nc.vector.tensor_add(sgb, gp, bp)

                # squares (ACT)
                r2 = plane_pool.tile([P, T], PLDT, tag="r2", name="r2")
                g2 = plane_pool.tile([P, T], PLDT, tag="g2", name="g2")
                b2 = plane_pool.tile([P, T], PLDT, tag="b2", name="b2")
                qrg = plane_pool.tile([P, T], PLDT, tag="qrg", name="qrg")
                qrb = plane_pool.tile([P, T], PLDT, tag="qrb", name="qrb")
                qgb = plane_pool.tile([P, T], PLDT, tag="qgb", name="qgb")
                nc.scalar.square(r2, rp)
                nc.scalar.square(g2, gp)
                nc.scalar.square(b2, bp)
                nc.scalar.square(qrg, srg)
                nc.scalar.square(qrb, srb)
                nc.scalar.square(qgb, sgb)

                basis = [rp, gp, bp, r2, g2, b2, qrg, qrb, qgb]

                y32 = px32_pool.tile([P, T, C], F32, tag="y32", name="y32")
                # products: DVE tensor_scalar (2x) for ACT_K-complement,
                # ACT scale-copies for ACT_K; combine with a tt-add tree.
                ACT_K = (3, 4, 5, 6)  # products computed on ScalarE
                for c in range(C):
                    u = []
                    for k in range(9):
                        uk = plane_pool.tile(
                            [P, T], PLDT, tag=f"u{k}", name=f"u{c}_{k}", 
                        )
                        if k in ACT_K:
                            nc.scalar.mul(uk, basis[k], ms(k, c))
                        else:
                            nc.vector.tensor_scalar(
                                uk, basis[k], ms(k, c), None, MULT
                            )
                        u.append(uk)
                    t0_ = plane_pool.tile([P, T], PLDT, tag="t0", name=f"t0_{c}")
                    t1_ = plane_pool.tile([P, T], PLDT, tag="t1", name=f"t1_{c}")
                    t2_ = plane_pool.tile([P, T], PLDT, tag="t2", name=f"t2_{c}")
                    t3_ = plane_pool.tile([P, T], PLDT, tag="t3", name=f"t3_{c}")
                    nc.vector.tensor_add(t0_, u[0], u[1])
                    nc.vector.tensor_add(t1_, u[2], u[3])
                    nc.vector.tensor_add(t2_, u[4], u[5])
                    nc.vector.tensor_add(t3_, u[6], u[7])
                    nc.vector.tensor_add(t0_, t0_, t1_)
                    nc.vector.tensor_add(t2_, t2_, t3_)
                    nc.vector.tensor_add(t0_, t0_, u[8])
                    nc.vector.tensor_add(y32[:, :, c], t0_, t2_)

                nc.sync.dma_start(
                    out=y_d[t].rearrange("p (t c) -> p t c", c=C), in_=y32
                )

    nc.compile()
    return nc


def _prep_inputs(x, histogram, W1, b1, W2, b2):
    """Host-side sharding / layout packing (no arithmetic on data except the
    static linear re-parameterization of the tiny weights W2/b2)."""
    x = np.ascontiguousarray(np.asarray(x, dtype=np.float32))
    hist = np.asarray(histogram, dtype=np.float32).reshape(B, HIST)
    W1 = np.ascontiguousarray(np.asarray(W1, dtype=np.float32))
    b1 = np.asarray(b1, dtype=np.float32)
    W2 = np.asarray(W2, dtype=np.float32)
    b2 = np.asarray(b2, dtype=np.float32)

    T27 = _coeff_transform()
    W2p = np.ascontiguousarray(
        (W2.astype(np.float64) @ T27.T).astype(np.float32).reshape(MT, P, MOUT)
    )
    b2p_flat = (b2.astype(np.float64) @ T27.T).astype(np.float32)

    w1_r = W1.reshape(KT, P, HID)
    b1rep = np.ascontiguousarray(np.broadcast_to(b1, (SPC, HID)))
    b2bc = np.ascontiguousarray(
        np.broadcast_to(np.tile(b2p_flat, SPC), (P, SPC * MOUT))
    )
    eye2 = np.eye(SPC, dtype=np.float32)

    in_maps = []
    for core in range(N_CORES):
        xs = x[core * SPC : (core + 1) * SPC].reshape(-1)
        x_core = np.ascontiguousarray(xs.reshape(NTILES, P, T * C))
        h_core = hist[core * SPC : (core + 1) * SPC]  # [SPC, HIST]
        hp = np.ascontiguousarray(
            h_core.reshape(SPC, KT, P).transpose(2, 1, 0).reshape(P, KT * SPC)
        )
        in_maps.append(
            {
                "x_core": x_core,
                "h_packed": hp,
                "w1": w1_r,
                "b1_rep": b1rep,
                "eye2": eye2,
                "w2p": W2p,
                "b2bc": b2bc,
            }
        )
    return in_maps


def run(trace=False, **inputs):
    if "nc" not in _CACHE:
        _CACHE["nc"] = _build()
    nc = _CACHE["nc"]
    in_maps = _prep_inputs(**inputs)
    res = bass_utils.run_bass_kernel_spmd(
        nc, in_maps, core_ids=list(range(N_CORES)), trace=trace
    )
    outs = np.stack([r["y_core"] for r in res.results])  # [8, NTILES, P, T*C]
    y = outs.reshape(B, H, W, C).astype(np.float32)
    return y, res


def kernel(**inputs) -> np.ndarray:
    y, _ = run(trace=False, **inputs)
    return y


if __name__ == "__main__":
    rng = np.random.default_rng(0)
    ins = {
        "x": rng.random((B, H, W, C), dtype=np.float32),
        "histogram": rng.random((B, 3, 64, 64), dtype=np.float32),
        "W1": (rng.standard_normal((HIST, HID)) / np.sqrt(HIST)).astype(np.float32),
        "b1": np.zeros(HID, np.float32),
        "W2": (rng.standard_normal((HID, MOUT)) / np.sqrt(HID)).astype(np.float32),
        "b2": np.zeros(MOUT, np.float32),
    }
    y = kernel(**ins)
    print("out", y.shape, y.dtype, float(np.abs(y).max()))


# revision 23
# speedup vs baseline: 1.0885x; 1.0389x over previous
"""AWBNet (wo R2) Trainium2 kernel.

Math (per sample b):
  m = reshape(relu(hist_flat @ W1 + b1) @ W2 + b2, [9, 3])
  feats(px) = [r, g, b, r^2, g^2, b^2, rg, rb, gb]
  y[px, c] = sum_k feats[px, k] * m[k, c]

Device strategy (8 cores, pure data parallel over batch, 2 samples/core):
  * Tiny MLP on TensorE in fp32 with natural layouts (host only re-packs
    histogram / b1 so no on-device transposes are needed).
  * Per-pixel einsum on VectorE/ScalarE in fp16 using the square basis
    {R, G, B, R^2, G^2, B^2, (R+G)^2, (R+B)^2, (G+B)^2}; the coefficient
    change (rg = ((R+G)^2 - R^2 - G^2)/2 etc.) is folded into W2/b2 on the
    host (pure linear re-parameterization of the weights, no data compute).
  * m-coefficients are broadcast to all 128 partitions by a fused
    matmul whose lhsT is a stride-0 (broadcast) column of featT; they are
    then per-partition scalars for the per-pixel products.
  * Per-pixel: ScalarE deinterleaves (stride-3 fp32 -> dense fp16) and
    squares; products m_k*F_k split DVE tensor_scalar (2x) / ACT
    scale-copies to balance the engines; DVE tt-add trees combine, the
    last add writing the stride-3 fp32 output view directly.
  * Three DMA queues in parallel: W1 stream + late x tiles on SWDGE
    (with fp32->fp16 cast), x0/x1 + y stores on the SP HWDGE ring, small
    setup DMAs on the ACT HWDGE ring.
"""

import sys

import numpy as np

for _p in ("/opt/trn_rl_repo",):
    if _p not in sys.path:
        sys.path.insert(0, _p)

import concourse.bacc as bacc
import concourse.mybir as mybir
import concourse.tile as tile
from concourse import bass_utils

# ---- problem constants (hardcoded per contract) ----
N_CORES = 8
B, H, W, C = 16, 512, 512, 3
SPC = B // N_CORES  # samples per core = 2
PX_SAMPLE = H * W  # 262144
PX_CORE = SPC * PX_SAMPLE  # 524288
P = 128
LANE_PX = PX_CORE // P  # 4096 pixels per partition per core
T = 1024  # pixels per partition per tile
NTILES = LANE_PX // T  # 4
TILES_PER_SAMPLE = NTILES // SPC  # 2

HIST = 3 * 64 * 64  # 12288
HID = 256
MOUT = 27
KT = HIST // P  # 96 k-tiles
MT = HID // P  # 2 m-tiles
W1_CH = 8  # k-tiles per W1 DMA chunk (8 * 128KB = 1MB)
KT_SH = KT // N_CORES  # 12 k-tiles of W1 per core (K-sharded MLP + AllReduce)

F16 = mybir.dt.float16
BF16 = mybir.dt.bfloat16
PLDT = mybir.dt.float16
F32 = mybir.dt.float32
MULT = mybir.AluOpType.mult
ADD = mybir.AluOpType.add
AF = mybir.ActivationFunctionType

_CACHE = {}


def _coeff_transform():
    """T27 so that m' = m_flat @ T27.T gives coefficients for the square
    basis [R,G,B,R2,G2,B2,(R+G)^2,(R+B)^2,(G+B)^2]."""
    T9 = np.zeros((9, 9), dtype=np.float64)
    for i in range(3):  # R,G,B linear terms pass through
        T9[i, i] = 1.0
    # new squares: old squares minus half the relevant cross terms
    # old order: 3=r2,4=g2,5=b2,6=rg,7=rb,8=gb
    T9[3, 3] = 1.0
    T9[3, 6] = -0.5
    T9[3, 7] = -0.5
    T9[4, 4] = 1.0
    T9[4, 6] = -0.5
    T9[4, 8] = -0.5
    T9[5, 5] = 1.0
    T9[5, 7] = -0.5
    T9[5, 8] = -0.5
    T9[6, 6] = 0.5  # (R+G)^2 coeff = rg/2
    T9[7, 7] = 0.5
    T9[8, 8] = 0.5
    T27 = np.zeros((27, 27), dtype=np.float64)
    for c in range(3):
        for kn in range(9):
            for ko in range(9):
                T27[3 * kn + c, 3 * ko + c] = T9[kn, ko]
    return T27


def _build():
    nc = bacc.Bacc(
        "TRN2", target_bir_lowering=False, debug=False, num_devices=N_CORES
    )

    x_d = nc.dram_tensor("x_core", [NTILES, P, T * C], F32, kind="ExternalInput")
    hp_d = nc.dram_tensor("h_packed", [P, KT * SPC], F32, kind="ExternalInput")
    w1_d = nc.dram_tensor("w1", [KT, P, HID], F32, kind="ExternalInput")
    b1_d = nc.dram_tensor("b1_rep", [SPC, HID], F32, kind="ExternalInput")
    w2_d = nc.dram_tensor("w2p", [MT, P, MOUT], F32, kind="ExternalInput")
    b2_d = nc.dram_tensor("b2bc", [P, SPC * MOUT], F32, kind="ExternalInput")
    eye_d = nc.dram_tensor("eye2", [SPC, SPC], F32, kind="ExternalInput")
    y_d = nc.dram_tensor("y_core", [NTILES, P, T * C], F32, kind="ExternalOutput")

    with tile.TileContext(nc) as tc:
        with (
            tc.tile_pool(name="mlp", bufs=1) as mlp_pool,
            tc.tile_pool(name="w1s", bufs=3) as w1_pool,
            tc.tile_pool(name="px32", bufs=2) as px32_pool,
            tc.tile_pool(name="pl16", bufs=2) as plane_pool,
            tc.tile_pool(name="ps", bufs=1, space="PSUM") as psum_pool,
        ):
            # ---------------- MLP (TensorE) ----------------
            hp_sb = mlp_pool.tile([P, KT * SPC], F16, tag="hp", name="hp")
            nc.gpsimd.dma_start(out=hp_sb, in_=hp_d[:, :])
            b1_sb = mlp_pool.tile([SPC, HID], F32, tag="b1", name="b1")
            nc.scalar.dma_start(out=b1_sb, in_=b1_d[:, :])
            w2_sb = mlp_pool.tile([P, MT, MOUT], F32, tag="w2", name="w2")
            nc.scalar.dma_start(out=w2_sb, in_=w2_d.rearrange("m p n -> p m n"))
            b2_sb = mlp_pool.tile([P, SPC * MOUT], F32, tag="b2", name="b2")
            nc.scalar.dma_start(out=b2_sb, in_=b2_d[:, :])
            eye_sb = mlp_pool.tile([SPC, SPC], F32, tag="eye", name="eye")
            nc.scalar.dma_start(out=eye_sb, in_=eye_d[:, :])

            # feat = h @ W1: lhsT = h-slices [128, 2] (cheap weight loads),
            # rhs = W1 k-tiles [128, 256] -> psum [2, 256] accumulated.
            feat_ps = psum_pool.tile([SPC, HID], F32, tag="featps", name="featps")
            for kc in range(KT // W1_CH):
                w1_sb = w1_pool.tile([P, W1_CH, HID], F16, tag="w1c", name="w1c")
                nc.gpsimd.dma_start(
                    out=w1_sb,
                    in_=w1_d[kc * W1_CH : (kc + 1) * W1_CH].rearrange(
                        "k p n -> p k n"
                    ),
                )
                for kk in range(W1_CH):
                    k = kc * W1_CH + kk
                    nc.tensor.matmul(
                        feat_ps,
                        hp_sb[:, k * SPC : (k + 1) * SPC],
                        w1_sb[:, kk, :],
                        start=(k == 0),
                        stop=(k == KT - 1),
                    )

            # relu(feat + b1) on DVE (b1 lives on the free dim here)
            feat_sb = mlp_pool.tile([SPC, HID], F32, tag="featsb", name="featsb")
            nc.vector.tensor_add(feat_sb, feat_ps, b1_sb)
            feat_r = mlp_pool.tile([SPC, HID], F32, tag="featr", name="featr")
            nc.vector.tensor_scalar(
                feat_r, feat_sb, 0.0, None, mybir.AluOpType.max
            )

            # transpose feat [2, 256] -> featT tiles [128, 2] via PE
            featT_sb = []
            for mt in range(MT):
                ft_ps = psum_pool.tile(
                    [P, SPC], F32, tag=f"ftps{mt}", name=f"ftps{mt}"
                )
                nc.tensor.transpose(
                    ft_ps, feat_r[:, mt * P : (mt + 1) * P], eye_sb
                )
                ft_sb = mlp_pool.tile(
                    [P, SPC], F32, tag=f"ftsb{mt}", name=f"ftsb{mt}"
                )
                nc.vector.tensor_copy(ft_sb, ft_ps)
                featT_sb.append(ft_sb)

            # fused m-matmul + partition-broadcast: a stride-0 lhsT column
            # makes every output partition compute m[s] = featT[:, s] @ W2'.
            mb_ps = psum_pool.tile([P, SPC * MOUT], F32, tag="mbps", name="mbps")
            for s in range(SPC):
                for mt in range(MT):
                    nc.tensor.matmul(
                        mb_ps[:, s * MOUT : (s + 1) * MOUT],
                        featT_sb[mt][:, s : s + 1].broadcast_to([P, P]),
                        w2_sb[:, mt, :],
                        start=(mt == 0),
                        stop=(mt == MT - 1),
                    )
            mscal = mlp_pool.tile([P, SPC * MOUT], F32, tag="mscal", name="mscal")
            nc.vector.tensor_add(mscal, mb_ps, b2_sb)

            # ---------------- pixel path ----------------
            for t in range(NTILES):
                s = t // TILES_PER_SAMPLE

                def ms(k, c, s=s):
                    j = s * MOUT + 3 * k + c
                    return mscal[:, j : j + 1]

                x32 = px32_pool.tile([P, T, C], F32, tag="x32", name="x32")
                x_dma = nc.sync if t < 2 else nc.gpsimd
                x_dma.dma_start(out=x32, in_=x_d[t].rearrange("p (t c) -> p t c", c=C))

                # deinterleave + cast to fp16 into channel-slices of one
                # wide [P, 3, T] tile (ACT, stride-3 reads)
                rgb = plane_pool.tile([P, C, T], PLDT, tag="rgb", name="rgb")
                nc.scalar.copy(rgb[:, 0, :], x32[:, :, 0])
                nc.scalar.copy(rgb[:, 1, :], x32[:, :, 1])
                nc.scalar.copy(rgb[:, 2, :], x32[:, :, 2])

                # pair sums (DVE fp16 2x) into a wide tile
                sm = plane_pool.tile([P, C, T], PLDT, tag="sm", name="sm")
                nc.vector.tensor_add(sm[:, 0, :], rgb[:, 0, :], rgb[:, 1, :])
                nc.vector.tensor_add(sm[:, 1, :], rgb[:, 0, :], rgb[:, 2, :])
                nc.vector.tensor_add(sm[:, 2, :], rgb[:, 1, :], rgb[:, 2, :])

                # squares: two wide ACT ops cover all six planes
                sq = plane_pool.tile([P, C, T], PLDT, tag="sq", name="sq")
                qq = plane_pool.tile([P, C, T], PLDT, tag="qq", name="qq")
                nc.scalar.square(sq, rgb)
                nc.scalar.square(qq, sm)

                basis = [
                    rgb[:, 0, :], rgb[:, 1, :], rgb[:, 2, :],
                    sq[:, 0, :], sq[:, 1, :], sq[:, 2, :],
                    qq[:, 0, :], qq[:, 1, :], qq[:, 2, :],
                ]

                y32 = px32_pool.tile([P, T, C], F32, tag="y32", name="y32")
                y32r = y32.rearrange("p t c -> p c t")
                # products: per-channel (distinct scalars) into channel-slices
                # of wide U tiles; adds: channel-merged [P, 3, T] tree.
                ACT_K = (3, 4, 5, 6)  # products computed on ScalarE

                def prods(k, uname):
                    uk = plane_pool.tile(
                        [P, C, T], PLDT, tag=uname, name=f"{uname}_{k}"
                    )
                    for c in range(C):
                        if k in ACT_K:
                            nc.scalar.mul(uk[:, c, :], basis[k], ms(k, c))
                        else:
                            nc.vector.tensor_scalar(
                                uk[:, c, :], basis[k], ms(k, c), None, MULT
                            )
                    return uk

                def tadd(tag, nm, a, b_):
                    o = plane_pool.tile([P, C, T], PLDT, tag=tag, name=nm)
                    nc.vector.tensor_add(o, a, b_)
                    return o

                ua = prods(0, "ua")
                ub = prods(1, "ub")
                ta1 = tadd("ta", f"ta1_{t}", ua, ub)
                ua = prods(2, "ua")
                ub = prods(3, "ub")
                tb1 = tadd("tb", f"tb1_{t}", ua, ub)
                tc1 = tadd("tc", f"tc1_{t}", ta1, tb1)
                ua = prods(4, "ua")
                ub = prods(5, "ub")
                ta2 = tadd("ta", f"ta2_{t}", ua, ub)
                ua = prods(6, "ua")
                ub = prods(7, "ub")
                tb2 = tadd("tb", f"tb2_{t}", ua, ub)
                ta3 = tadd("ta", f"ta3_{t}", ta2, tb2)
                ua = prods(8, "ua")
                tc2 = tadd("tc", f"tc2_{t}", tc1, ua)
                nc.vector.tensor_add(y32r, ta3, tc2)

                nc.sync.dma_start(
                    out=y_d[t].rearrange("p (t c) -> p t c", c=C), in_=y32
                )

    nc.compile()
    return nc


def _prep_inputs(x, histogram, W1, b1, W2, b2):
    """Host-side sharding / layout packing (no arithmetic on data except the
    static linear re-parameterization of the tiny weights W2/b2)."""
    x = np.ascontiguousarray(np.asarray(x, dtype=np.float32))
    hist = np.asarray(histogram, dtype=np.float32).reshape(B, HIST)
    W1 = np.ascontiguousarray(np.asarray(W1, dtype=np.float32))
    b1 = np.asarray(b1, dtype=np.float32)
    W2 = np.asarray(W2, dtype=np.float32)
    b2 = np.asarray(b2, dtype=np.float32)

    T27 = _coeff_transform()
    W2p = np.ascontiguousarray(
        (W2.astype(np.float64) @ T27.T).astype(np.float32).reshape(MT, P, MOUT)
    )
    b2p_flat = (b2.astype(np.float64) @ T27.T).astype(np.float32)

    w1_r = W1.reshape(KT, P, HID)
    b1rep = np.ascontiguousarray(np.broadcast_to(b1, (SPC, HID)))
    b2bc = np.ascontiguousarray(
        np.broadcast_to(np.tile(b2p_flat, SPC), (P, SPC * MOUT))
    )
    eye2 = np.eye(SPC, dtype=np.float32)

    in_maps = []
    for core in range(N_CORES):
        xs = x[core * SPC : (core + 1) * SPC].reshape(-1)
        x_core = np.ascontiguousarray(xs.reshape(NTILES, P, T * C))
        h_core = hist[core * SPC : (core + 1) * SPC]  # [SPC, HIST]
        hp = np.ascontiguousarray(
            h_core.reshape(SPC, KT, P).transpose(2, 1, 0).reshape(P, KT * SPC)
        )
        in_maps.append(
            {
                "x_core": x_core,
                "h_packed": hp,
                "w1": w1_r,
                "b1_rep": b1rep,
                "eye2": eye2,
                "w2p": W2p,
                "b2bc": b2bc,
            }
        )
    return in_maps


def run(trace=False, **inputs):
    if "nc" not in _CACHE:
        _CACHE["nc"] = _build()
    nc = _CACHE["nc"]
    in_maps = _prep_inputs(**inputs)
    res = bass_utils.run_bass_kernel_spmd(
        nc, in_maps, core_ids=list(range(N_CORES)), trace=trace
    )
    outs = np.stack([r["y_core"] for r in res.results])  # [8, NTILES, P, T*C]
    y = outs.reshape(B, H, W, C).astype(np.float32)
    return y, res


def kernel(**inputs) -> np.ndarray:
    y, _ = run(trace=False, **inputs)
    return y


if __name__ == "__main__":
    rng = np.random.default_rng(0)
    ins = {
        "x": rng.random((B, H, W, C), dtype=np.float32),
        "histogram": rng.random((B, 3, 64, 64), dtype=np.float32),
        "W1": (rng.standard_normal((HIST, HID)) / np.sqrt(HIST)).astype(np.float32),
        "b1": np.zeros(HID, np.float32),
        "W2": (rng.standard_normal((HID, MOUT)) / np.sqrt(HID)).astype(np.float32),
        "b2": np.zeros(MOUT, np.float32),
    }
    y = kernel(**ins)
    print("out", y.shape, y.dtype, float(np.abs(y).max()))
